# revision 28
# baseline (speedup 1.0000x reference)
"""MCCDecoderAttention Trainium2 kernel (8 NeuronCores).

Sharding: core = b*4 + g  (b in {0,1} batch, g in {0..3} head-group).
Each core computes attention for 3 heads of one batch plus its partial
contribution to the output projection; the host sums the 4 partials per
batch and adds b_proj.

Layout (all operands bf16, f32 PSUM accumulation):
  xT    [768, 2048]  x[b].T (feature-major)
  wqkT  [768, 384]   w_qkv.T columns [q_h0 q_h1 | k_h0 k_h1 | q_h2 k_h2]
  wvT   [768, 192]   w_qkv.T v-columns [v_h0 v_h1 v_h2]
  wpT   [192, 768]   w_proj.T rows for this core's 192 channels
  yT    [768, 2048]  partial output (feature-major, f32)

Single fused pipeline: chunked input DMA -> per-chunk Q/K/V projection
-> transposed-S attention (keys on partitions, queries on the free axis;
exp on the Act engine with the 1/8 scale folded in; A@V accumulates over
key tiles with a ones-column spliced into the per-head V block so the
softmax denominator L falls out of the same matmuls) -> per-chunk
normalization -> 2-pass output projection.  Head 0's V block is laid out
[V | ones] so its attention output lands on PSUM rows 0-63 while heads
1/2 use [ones | 0*63 | V] landing on rows 64-127: heads 0+1 pack into
one [128 x N] tile and the projection contracts 128+64 instead of
3 x 64.  L-row broadcasts run on the idle GpSimd/Pool engine (which can
only read absolute partition 0 - hence L-on-row-0 for heads 1/2) or as
a 1-row PE matmul for head 0 (L on row 64).  Projection and leftover
QKV matmuls are interleaved into the attention phase as PE filler so
the Tensor engine never idles behind the Act-engine exp stream.  The
decoder mask (last `unseen` keys masked except the diagonal) is handled
by looping keys over [0, N-u) only, plus an elementwise diagonal
correction for queries in the unseen range.
"""

import functools
import os
import sys

for _p in ("/opt/trn_rl_repo", "/root/.axon_site/_ro/trn_rl_repo"):
    if os.path.isdir(_p) and _p not in sys.path:
        sys.path.insert(0, _p)

import numpy as np

import concourse.bacc as bacc
import concourse.tile as tile
from concourse import mybir

N, C, D = 2048, 768, 64
NH = 3            # heads per core
CT = C // 128     # 6 contraction tiles
F32 = mybir.dt.float32
F32R = mybir.dt.float32r
BF = mybir.dt.bfloat16
EXP = mybir.ActivationFunctionType.Exp

_last_results = None  # BassKernelResults of the most recent run (for test.py)


@functools.lru_cache(maxsize=4)
def _build(u: int):
    nc = bacc.Bacc(None, target_bir_lowering=False)
    xT = nc.dram_tensor("xT", [C, N], BF, kind="ExternalInput")
    wqkT = nc.dram_tensor("wqkT", [C, 384], BF, kind="ExternalInput")
    wvT = nc.dram_tensor("wvT", [C, 192], BF, kind="ExternalInput")
    wpT = nc.dram_tensor("wpT", [NH * D, C], BF, kind="ExternalInput")
    yT = nc.dram_tensor("yT", [C, N], F32, kind="ExternalOutput")

    kfull = N - u
    t_full, rem = divmod(kfull, 128)
    T = t_full + (1 if rem else 0)
    fast = (u == 512)  # tuned filler schedule for the benched shape

    with nc.allow_low_precision(reason="bf16 attention staging"), \
         tile.TileContext(nc) as tc, \
         tc.tile_pool(name="persist", bufs=1) as P, \
         tc.tile_pool(name="scr", bufs=2) as S, \
         tc.tile_pool(name="apool", bufs=14) as A, \
         tc.tile_pool(name="opool", bufs=4) as O, \
         tc.tile_pool(name="stp", bufs=2, space="PSUM") as STP, \
         tc.tile_pool(name="avp", bufs=2, space="PSUM") as AVP, \
         tc.tile_pool(name="auxp", bufs=2, space="PSUM") as AUX:
        xt = P.tile([128, CT, N], BF)
        wqk = P.tile([128, CT, 384], BF)
        wv = P.tile([128, CT, 192], BF)
        wpP = P.tile([128, C], BF)   # w_proj rows: head 0 at 0:64, head 1 at 64:128
        wp2 = P.tile([128, C], BF)   # head 2 rows at partitions 64-127
        qT0 = P.tile([128, N], BF)
        kT0 = P.tile([128, N], BF)
        qT1 = P.tile([64, N], BF)
        kT1 = P.tile([64, N], BF)
        # per-head 128-wide V blocks (see module docstring):
        #   h0: [V(0:64) | ones(64) | -]        -> av rows: O 0-63, L 64
        #   h1/h2: [ones(0) | 0*63 | V(64:128)] -> av rows: L 0, O 64-127
        vsb = P.tile([128, T, NH, 128], BF)
        aoP = P.tile([128, N], BF)   # normalized attn out, heads 0 + 1
        ao2 = P.tile([128, N], BF)   # head 2 at rows 64-127
        ones = P.tile([128, 64], BF)
        onesf = P.tile([128, 64], F32R)
        zs = P.tile([128, 1], F32)
        # vtuAB: h0 rows 0-63, h1 rows 64-127; vtuC: h2 rows 64-127
        vtuAB = P.tile([128, u], F32, name="vtuAB") if u else None
        vtuC = P.tile([128, u], F32, name="vtuC") if u else None
        est = [S.tile([128, 512], F32R, tag=f"est{_h}", bufs=1,
                      name=f"est{_h}") for _h in range(NH)] if u else []

        nc.gpsimd.memset(ones[:], 1.0)
        ones_f = P.tile([128, 64], F32)
        nc.gpsimd.memset(ones_f[:], 1.0)
        nc.vector.tensor_copy(onesf[:], ones_f[:])
        nc.gpsimd.memset(zs[:], 0.0)
        nc.gpsimd.memset(vsb[:], 0.0)
        nc.gpsimd.memset(vsb[:, :, 0, 64:65], 1.0)
        nc.gpsimd.memset(vsb[:, :, 1:3, 0:1], 1.0)

        # ---- input DMAs: x + wqk on the SP queue, other weights on the
        # Act queue so the transfers overlap; ordered by first use ----
        xTr = xT.rearrange("(t p) n -> p t n", p=128)
        wqkr = wqkT.rearrange("(t p) f -> p t f", p=128)
        wvr = wvT.rearrange("(t p) f -> p t f", p=128)
        nc.sync.dma_start(xt[:, 0:3, 0:512], xTr[:, 0:3, 0:512])
        nc.sync.dma_start(wqk[:, 0:3, :], wqkr[:, 0:3, :])
        nc.sync.dma_start(xt[:, 3:6, 0:512], xTr[:, 3:6, 0:512])
        nc.sync.dma_start(wqk[:, 3:6, :], wqkr[:, 3:6, :])
        nc.sync.dma_start(wv[:], wvr[:])
        nc.sync.dma_start(xt[:, :, 512:1024], xTr[:, :, 512:1024])
        nc.sync.dma_start(xt[:, :, 1024:1536], xTr[:, :, 1024:1536])
        nc.sync.dma_start(xt[:, :, 1536:2048], xTr[:, :, 1536:2048])
        nc.gpsimd.dma_start(wpP[:], wpT[0:128, :])
        nc.gpsimd.dma_start(wp2[64:128, :], wpT[128:192, :])

        # ---- closure factories (each = one PSUM-tile of PE work) ----
        def qkproj(ch, fi):
            def f():
                sl = slice(ch * 512, (ch + 1) * 512)
                ps = AUX.tile([128, 512], F32, tag="aux", name="qkps")
                for ct in range(CT):
                    nc.tensor.matmul(ps[:], wqk[:, ct, fi * 128:(fi + 1) * 128],
                                     xt[:, ct, sl], start=(ct == 0),
                                     stop=(ct == CT - 1), skip_group_check=True)
                if fi == 0:
                    nc.vector.tensor_copy(qT0[:, sl], ps[:])
                elif fi == 1:
                    nc.vector.tensor_copy(kT0[:, sl], ps[:])
                else:
                    nc.vector.tensor_copy(qT1[:, sl], ps[0:64, :])
                    nc.vector.tensor_copy(kT1[:, sl], ps[64:128, :])
            return f

        def vproj(nt):
            def f():
                ps = AUX.tile([128, 192], F32, tag="aux", name="vps")
                for ct in range(CT):
                    nc.tensor.matmul(ps[:], xt[:, ct, nt * 128:(nt + 1) * 128],
                                     wv[:, ct, :], start=(ct == 0),
                                     stop=(ct == CT - 1), skip_group_check=True)
                nc.vector.tensor_copy(vsb[:, nt, 0, 0:64], ps[:, 0:64])
                nc.vector.tensor_copy(
                    vsb[:, nt, 1:3, 64:128],
                    ps[:, 64:192].rearrange("p (h x) -> p h x", x=64))
            return f

        def vtuproj(part):
            # part 0: heads 0+1 in one [128, w] pass; part 1: head 2 at base 64
            def f():
                for uc in range(0, u, 512):
                    w = min(512, u - uc)
                    ps = AUX.tile([128, 512], F32, tag="aux", name="vtups")
                    if part == 0:
                        for ct in range(CT):
                            nc.tensor.matmul(ps[:, 0:w], wv[:, ct, 0:128],
                                             xt[:, ct, kfull + uc:kfull + uc + w],
                                             start=(ct == 0),
                                             stop=(ct == CT - 1),
                                             skip_group_check=True)
                        nc.vector.tensor_copy(vtuAB[:, uc:uc + w], ps[:, 0:w])
                    else:
                        for ct in range(CT):
                            nc.tensor.matmul(ps[64:128, 0:w],
                                             wv[:, ct, 128:192],
                                             xt[:, ct, kfull + uc:kfull + uc + w],
                                             start=(ct == 0),
                                             stop=(ct == CT - 1),
                                             skip_group_check=True)
                        nc.vector.tensor_copy(vtuC[64:128, uc:uc + w],
                                              ps[64:128, 0:w])
            return f

        def head_qkv(h):
            if h < 2:
                return (qT0[h * 64:(h + 1) * 64, :],
                        kT0[h * 64:(h + 1) * 64, :], h * 64)
            return qT1[0:64, :], kT1[0:64, :], 0

        # precomputable diagonal terms (fast path): est[h] holds
        # exp(q_i . k_i / 8) for i in [kfull, N) on row Lr
        def diag_pre(h):
            Lr = 64 if h == 0 else 0
            qv, kv, bh = head_qkv(h)

            def f():
                prod = S.tile([128, 512], BF, tag="prod", bufs=1, name="prod")
                nc.vector.tensor_mul(prod[bh:bh + 64, 0:u],
                                     qv[:, kfull:N], kv[:, kfull:N])
                dg = AUX.tile([128, 512], F32, tag="aux", name="dg")
                nc.tensor.matmul(dg[Lr:Lr + 64, 0:u], ones[bh:bh + 64, :],
                                 prod[bh:bh + 64, 0:u], start=True,
                                 stop=True, skip_group_check=True)
                nc.scalar.activation(est[h][Lr:Lr + 1, 0:u],
                                     dg[Lr:Lr + 1, 0:u], EXP,
                                     bias=zs[Lr:Lr + 1, :], scale=0.125)
            return f

        def proj(qc, co, pool=None, eng=None):
            def f():
                sl = slice(qc * 512, (qc + 1) * 512)
                pl = pool if pool is not None else AUX
                ps = pl.tile([128, 512], F32,
                             tag="aux" if pl is AUX else "av", name="pjps")
                nc.tensor.matmul(ps[:], wpP[:, co * 128:(co + 1) * 128],
                                 aoP[:, sl], start=True, stop=False,
                                 skip_group_check=True)
                nc.tensor.matmul(ps[:], wp2[64:128, co * 128:(co + 1) * 128],
                                 ao2[64:128, sl], start=False, stop=True,
                                 skip_group_check=True)
                o = O.tile([128, 512], F32, tag="o", name="o")
                if eng == "s":
                    nc.scalar.copy(o[:], ps[:])
                else:
                    nc.vector.tensor_copy(o[:], ps[:])
                nc.sync.dma_start(yT[co * 128:(co + 1) * 128, sl], o[:])
            return f

        # ---- normalization (per 512-query chunk) ----
        # av rows: L at Lr, O at [ob, ob+64).  The av PSUM slot is released
        # by a single copy to SBUF so the next block's A@V can start.
        def normalize(p2, h, cc, av):
            qv, kv, bh = head_qkv(h)
            Lr = 64 if h == 0 else 0
            ob = 0 if h == 0 else 64
            dst = aoP if h < 2 else ao2
            qs = p2 * 1024 + cc * 512
            qe = qs + 512
            sl = slice(qs, qe)
            us = max(qs, kfull)
            off = us - qs
            masked = us < qe and u > 0
            if masked:
                if fast:
                    es, eoff = est[h], us - kfull
                else:
                    es, eoff = S.tile([128, 512], F32R, tag="esg", bufs=1,
                                      name="esg"), off
                    prod = S.tile([128, 512], BF, tag="prod", bufs=1,
                                  name="prod")
                    nc.vector.tensor_mul(prod[bh:bh + 64, off:512],
                                         qv[:, us:qe], kv[:, us:qe])
                    dg = AUX.tile([128, 512], F32, tag="aux", name="dg")
                    nc.tensor.matmul(dg[Lr:Lr + 64, off:512],
                                     ones[bh:bh + 64, :],
                                     prod[bh:bh + 64, off:512], start=True,
                                     stop=True, skip_group_check=True)
                    nc.scalar.activation(es[Lr:Lr + 1, off:512],
                                         dg[Lr:Lr + 1, off:512], EXP,
                                         bias=zs[Lr:Lr + 1, :], scale=0.125)
                # L += exp(diag) in place on the PSUM L row
                nc.vector.tensor_add(av[Lr:Lr + 1, off:512],
                                     av[Lr:Lr + 1, off:512],
                                     es[Lr:Lr + 1, eoff:eoff + qe - us])
            rr = S.tile([128, 512], F32R, tag="rrow", name="rr")
            nc.vector.reciprocal(rr[Lr:Lr + 1, :], av[Lr:Lr + 1, :])
            if h == 0:
                bl = AUX.tile([128, 512], F32, tag="aux", name="bl")
                nc.tensor.matmul(bl[ob:ob + 64, :], onesf[Lr:Lr + 1, 0:64],
                                 rr[Lr:Lr + 1, :], start=True, stop=True,
                                 skip_group_check=True)
                rbv = S.tile([128, 512], F32R, tag="rbc", name="rbv")
                nc.vector.tensor_copy(rbv[ob:ob + 64, :], bl[ob:ob + 64, :])
            else:
                rbv = S.tile([128, 512], F32R, tag="rbc", name="rbv")
                nc.gpsimd.partition_broadcast(rbv[:, :], rr[Lr:Lr + 1, :])
            nc.vector.tensor_mul(dst[ob:ob + 64, sl], av[ob:ob + 64, :],
                                 rbv[ob:ob + 64, :])
            if masked:
                # add back exp(diag)/L * v_i on the diagonal block
                er = S.tile([128, 512], F32R, tag="erow", bufs=1, name="er")
                nc.vector.tensor_mul(er[Lr:Lr + 1, off:512],
                                     es[Lr:Lr + 1, eoff:eoff + qe - us],
                                     rr[Lr:Lr + 1, off:512])
                if h == 0:
                    ebl = AUX.tile([128, 512], F32, tag="aux", name="ebl")
                    nc.tensor.matmul(ebl[ob:ob + 64, off:512],
                                     onesf[Lr:Lr + 1, 0:64],
                                     er[Lr:Lr + 1, off:512], start=True,
                                     stop=True, skip_group_check=True)
                    ebv = ebl
                else:
                    eb = S.tile([128, 512], F32R, tag="ebc", bufs=1, name="eb")
                    nc.gpsimd.partition_broadcast(eb[:, off:512],
                                                  er[Lr:Lr + 1, off:512])
                    ebv = eb
                tm = S.tile([128, 512], F32, tag="tmpc", bufs=1, name="tm")
                vts = vtuAB if h < 2 else vtuC
                nc.vector.tensor_mul(tm[ob:ob + 64, off:512],
                                     vts[ob:ob + 64, us - kfull:qe - kfull],
                                     ebv[ob:ob + 64, off:512])
                nc.vector.tensor_add(dst[ob:ob + 64, us:qe],
                                     dst[ob:ob + 64, us:qe],
                                     tm[ob:ob + 64, off:512])

        # ---- attention block: one (query-half, head) pair ----
        # A block = one (query-half, head) pair.  start_block issues the
        # first two S^T tiles (+ their exps); run_block runs the A@V loop
        # with fillers, starts the NEXT block's S^T before the normalize
        # chains so the in-order PE stream never waits on DVE/Pool work at
        # block boundaries, then normalizes.
        def issue_st(qv, kv, q0, pend, t):
            st = STP.tile([128, 1024], F32, tag="st", name="st")
            for cc in range(2):
                nc.tensor.matmul(st[:, cc * 512:(cc + 1) * 512],
                                 kv[:, t * 128:t * 128 + 128],
                                 qv[:, q0 + cc * 512:q0 + cc * 512 + 512],
                                 start=True, stop=True,
                                 skip_group_check=True)
            a = A.tile([128, 1024], BF, tag="a", name="a")
            nc.scalar.activation(a[:], st[:], EXP, scale=0.125)
            if t == T - 1 and rem:
                nc.vector.memset(a[rem:128, :], 0.0)
            pend[t] = a

        def start_block(p2, h):
            qv, kv, bh = head_qkv(h)
            q0 = p2 * 1024
            avs = [AVP.tile([128, 512], F32, name=f"av{_c}", tag="av")
                   for _c in range(2)]
            pend = {}
            issue_st(qv, kv, q0, pend, 0)
            if T > 1:
                issue_st(qv, kv, q0, pend, 1)
            return (p2, h, qv, kv, q0, avs, pend)

        def run_block(state, fillers, nxt):
            p2, h, qv, kv, q0, avs, pend = state
            vw = 65 if h == 0 else 128  # lhsT width of the per-head V block
            for t in range(T):
                a = pend.pop(t)
                for cc in range(2):
                    nc.tensor.matmul(avs[cc][0:vw, :], vsb[:, t, h, 0:vw],
                                     a[:, cc * 512:(cc + 1) * 512],
                                     start=(t == 0), stop=(t == T - 1),
                                     skip_group_check=True)
                if t + 2 < T:
                    issue_st(qv, kv, q0, pend, t + 2)
                if t < len(fillers) and fillers[t] is not None:
                    fillers[t]()
            for f in fillers[T:]:
                if f is not None:
                    f()
            nstate = start_block(*nxt) if nxt else None
            if h != 0 and nxt:
                # defer the (PE-free) normalize chains into the next block's
                # first filler slots: the A@V av slots release while the next
                # block's matmuls run, instead of stalling its start
                posts = [(lambda cc: lambda: normalize(p2, h, cc, avs[cc]))(c)
                         for c in range(2)]
            else:
                for cc in range(2):
                    normalize(p2, h, cc, avs[cc])
                posts = []
            return nstate, posts

        # First block runs its two query chunks cc-sequentially with
        # half-width S^T tiles: attention (and the Act exp stream) starts
        # right after the ch0 Q/K projection instead of waiting for ch1.
        def run_block0(fillersA, fillersB, nxt):
            qv, kv, bh = head_qkv(0)
            avs = [AVP.tile([128, 512], F32, name=f"av{_c}", tag="av")
                   for _c in range(2)]
            pend = {}

            def ist(cc, t):
                st = STP.tile([128, 512], F32, tag="st", name="sth")
                nc.tensor.matmul(st[:], kv[:, t * 128:t * 128 + 128],
                                 qv[:, cc * 512:cc * 512 + 512],
                                 start=True, stop=True, skip_group_check=True)
                a = A.tile([128, 1024], BF, tag="a", name="a")
                nc.scalar.activation(a[:, 0:512], st[:], EXP, scale=0.125)
                if t == T - 1 and rem:
                    nc.vector.memset(a[rem:128, 0:512], 0.0)
                pend[(cc, t)] = a

            def run_fill(fl, t):
                if t < len(fl) and fl[t] is not None:
                    fs = fl[t] if isinstance(fl[t], list) else [fl[t]]
                    for f in fs:
                        f()

            for cc in range(2):
                fl = fillersA if cc == 0 else fillersB
                ist(cc, 0)
                if T > 1:
                    ist(cc, 1)
                for t in range(T):
                    run_fill(fl, t)  # fillers first: they feed later k/v tiles
                    a = pend.pop((cc, t))
                    nc.tensor.matmul(avs[cc][0:65, :], vsb[:, t, 0, 0:65],
                                     a[:, 0:512], start=(t == 0),
                                     stop=(t == T - 1),
                                     skip_group_check=True)
                    if t + 2 < T:
                        ist(cc, t + 2)
                for t in range(T, len(fl)):
                    run_fill(fl, t)
            nstate = start_block(*nxt)
            normalize(0, 0, 0, avs[0])
            normalize(0, 0, 1, avs[1])
            return nstate

        # Final block runs its two query chunks cc-sequentially: chunk 0's
        # A@V finishes while chunk 1's is still accumulating, so chunk 0's
        # normalize + projection hide under chunk 1's matmuls.
        def run_block_last(state, fillersA, fillersB, tail1):
            p2, h, qv, kv, q0, avs, pend = state
            vw = 65 if h == 0 else 128
            for t in range(T):
                nc.tensor.matmul(avs[0][0:vw, :], vsb[:, t, h, 0:vw],
                                 pend[t][:, 0:512],
                                 start=(t == 0), stop=(t == T - 1),
                                 skip_group_check=True)
                if t + 2 < T:
                    issue_st(qv, kv, q0, pend, t + 2)
                if t < len(fillersA) and fillersA[t] is not None:
                    fillersA[t]()
            for f in fillersA[T:]:
                if f is not None:
                    f()
            normalize(p2, h, 0, avs[0])
            for t in range(T):
                a = pend.pop(t)
                nc.tensor.matmul(avs[1][0:vw, :], vsb[:, t, h, 0:vw],
                                 a[:, 512:1024],
                                 start=(t == 0), stop=(t == T - 1),
                                 skip_group_check=True)
                if t < len(fillersB) and fillersB[t] is not None:
                    fillersB[t]()
            normalize(p2, h, 1, avs[1])
            for f in tail1:
                f()

        # ---- schedule ----
        # Block order ends on a p2=0 block so the expensive masked-chunk
        # normalize chains of (1, h) blocks hide inside later blocks, and
        # the p2=1 projections run as fillers in the final block; only the
        # cheap p2=0 projections remain in the tail.
        if fast:
            # inline just enough for the first half-block (q/k ch0 + v0);
            # the rest of phase 1 rides along as b0 fillers
            qkproj(0, 0)()
            qkproj(0, 1)()
            vproj(0)()
            blocks = [(0, 0), (1, 0), (0, 1), (1, 1), (1, 2), (0, 2)]
            fillersA0 = [qkproj(1, 1), vproj(1), vproj(2), vproj(3),
                         vproj(4), vproj(5), [qkproj(2, 1), vproj(6)],
                         vproj(7), vproj(8), [qkproj(1, 0), vproj(9)],
                         vproj(10), vproj(11)]
            fillersB0 = [qkproj(0, 2), qkproj(1, 2), qkproj(2, 0),
                         qkproj(3, 0), None]
            fillers = [
                None,
                [qkproj(3, 1), vtuproj(0), diag_pre(0)],
                [qkproj(2, 2), qkproj(3, 2), diag_pre(1)],
                [vtuproj(1), diag_pre(2)],
                [],
                [proj(2, 0), proj(2, 1), proj(2, 2), proj(2, 3),
                 proj(2, 4), proj(2, 5), proj(3, 0), proj(3, 1),
                 proj(3, 2), proj(3, 3), proj(3, 4), proj(3, 5)],
            ]
            fillersB = [None, None, None, None,
                        proj(0, 0), proj(0, 1, eng="s"), proj(0, 2),
                        proj(0, 3, eng="s"), proj(0, 4), proj(0, 5, eng="s"),
                        None, None]
            tail1 = [proj(1, co,
                          pool=(AVP if co % 2 else AUX),
                          eng=("s" if co % 2 else "v"))
                     for co in range(CT)]
        else:
            # conservative general-u path: all projections before attention
            for ch in range(4):
                for fi in range(3):
                    qkproj(ch, fi)()
            for nt in range(T):
                vproj(nt)()
            if u:
                vtuproj(0)()
                vtuproj(1)()
            blocks = [(0, 0), (0, 1), (0, 2), (1, 0), (1, 1), (1, 2)]
            half = [proj(qc, co) for qc in (0, 1) for co in range(CT)]
            fillers = [[], [], [], half[0:4], half[4:8], half[8:12]]
            tail = [proj(qc, co) for qc in (2, 3) for co in range(CT)]

        posts = []
        if fast:
            state = run_block0(fillersA0, fillersB0, blocks[1])
            for bi in range(1, len(blocks) - 1):
                state, posts = run_block(state, posts + fillers[bi],
                                         blocks[bi + 1])
            run_block_last(state, posts + fillers[-1], fillersB, tail1)
        else:
            state = start_block(*blocks[0])
            for bi in range(len(blocks)):
                nxt = blocks[bi + 1] if bi + 1 < len(blocks) else None
                state, posts2 = run_block(state, posts + fillers[bi], nxt)
                posts = posts2
            for f in posts:
                f()
            for f in tail:
                f()

    nc.compile()
    return nc


def kernel(**inputs):
    global _last_results
    from concourse.bass_utils import run_bass_kernel_spmd
    import ml_dtypes

    BFNP = ml_dtypes.bfloat16
    x = np.asarray(inputs["x"], np.float32)
    w_qkv = np.asarray(inputs["w_qkv"], np.float32)
    w_proj = np.asarray(inputs["w_proj"], np.float32)
    b_proj = np.asarray(inputs["b_proj"], np.float32)
    u = int(np.asarray(inputs["unseen_size"]))
    B = x.shape[0]

    nc = _build(u)

    wT = np.ascontiguousarray(w_qkv.T)         # [768, 2304]
    wpT_full = np.ascontiguousarray(w_proj.T)  # [768, 768] (ci, co)
    xTb = [np.ascontiguousarray(x[b].T).astype(BFNP) for b in range(B)]

    in_maps = []
    for core in range(8):
        b, g = divmod(core, 4)
        hs = [3 * g, 3 * g + 1, 3 * g + 2]
        qcols = [0 * C + h * D + i for h in hs[:2] for i in range(D)]
        kcols = [1 * C + h * D + i for h in hs[:2] for i in range(D)]
        q2 = [0 * C + hs[2] * D + i for i in range(D)]
        k2 = [1 * C + hs[2] * D + i for i in range(D)]
        vcols = [2 * C + h * D + i for h in hs for i in range(D)]
        wqkTc = np.ascontiguousarray(
            wT[:, qcols + kcols + q2 + k2]).astype(BFNP)
        wvTc = np.ascontiguousarray(wT[:, vcols]).astype(BFNP)
        ci = [h * D + i for h in hs for i in range(D)]
        wpTc = np.ascontiguousarray(wpT_full[ci, :]).astype(BFNP)
        in_maps.append({"xT": xTb[b], "wqkT": wqkTc, "wvT": wvTc,
                        "wpT": wpTc})

    trace = bool(int(os.environ.get("KERNEL_TRACE", "0")))
    res = run_bass_kernel_spmd(nc, in_maps, core_ids=list(range(8)),
                               trace=trace)
    _last_results = res

    y = np.zeros((B, N, C), np.float32)
    for core in range(8):
        b = core // 4
        y[b] += res.results[core]["yT"].T
    y += b_proj
    return y
